# revision 13
# baseline (speedup 1.0000x reference)
"""Distributed causal attention kernel for one TRN2 chip (8 NeuronCores).

Problem: out = (softmax_causal((xWq)(xWk)^T / sqrt(dh)) (xWv)) Wout + b
  N=8192, D_IN=1024, D_HEAD=128, D_OUT=1024, fp32 I/O (bf16/fp8 compute).

Sharding (zig-zag for causal load balance): the sequence is split into
16 chunks of 512 rows; core c owns chunks c and 15-c, so every core has
the same causal attention area (17 blocks of 512x512).  Q stays local,
K/V shards are computed locally and AllGather'ed (bf16).

Layout: scores are computed transposed, St[j, i] = K Q^T, so that the
softmax-weighted PV matmul needs no transposes: O^T[dh, i] = V^T P^T via
lhsT = V (natural), rhs = exp(St).  Softmax skips the max-subtraction
(scores are ~N(0,1), |s| < ~7) and defers normalization: the row-sum is
accumulated with a ones-vector matmul and the division happens after
the output projection.

Scheduling: the gpsimd queue carries ONLY the collective instructions,
so the NRT entry barrier starts as early as possible and overlaps the
projections.  Input loads are spread across the sync/scalar DMA queues
in per-chunk pieces; weights arrive pre-cast to bf16 and the causal
mask is a host constant, so no staging copies block the start.  The
attention items are software-pipelined (item k's score matmuls + exps
are emitted before item k-1's PV/rowsum) so the tensor queue never
head-of-line blocks on an exp in flight.  The scalar engine does only
exps; the epilogue scaling runs on the vector engine.

SPMD uniformity: all cores run one program.  Of the 17 causal work
items per core, 2 are the diagonal blocks (local k/v, computed while
the all-gather is in flight), 8 are statically identical across cores,
and 7 select their (q-half, kv-block) via DVE registers derived from
partition_id and dynamic `ds()` slices, with PV partials accumulated
into an SBUF accumulator by the vector engine.
"""

import sys

import numpy as np

if "/opt/trn_rl_repo" not in sys.path:
    sys.path.insert(0, "/opt/trn_rl_repo")

import concourse.mybir as mybir
import concourse.tile as tile
from concourse import bacc
from concourse.bass import ds

F32 = mybir.dt.float32
BF16 = mybir.dt.bfloat16
F8 = mybir.dt.float8e4
AF = mybir.ActivationFunctionType
ALU = mybir.AluOpType
DR = mybir.MatmulPerfMode.DoubleRow
EXP_BIAS = -1.5


def build_program(cores=8, n=8192, d_in=1024, d_out=1024, dh=128,
                  enable_asserts=False, skip_bias=False):
    nchunk = 2 * cores            # zig-zag chunks
    ch = n // nchunk              # rows per chunk (512)
    r = 2 * ch                    # rows per core (1024)
    kd = d_in // 128              # contraction chunks for projections
    sub = ch // 128               # 128-row sub-chunks per kv block
    it = ch // 128                # 128-row i-tiles per half
    scale = float(dh) ** -0.5
    sw = sub * ch                 # score tile width (free elems per item)
    m_t = 512 if d_out >= 512 else d_out   # out-proj moving width
    mh = d_out // m_t
    gs = max(1, sub // 2)         # subchunks per St group (double-buffer)

    nc = bacc.Bacc("TRN2", target_bir_lowering=False, debug=False,
                   num_devices=cores, enable_asserts=enable_asserts)

    xT = nc.dram_tensor("xT", [d_in, r], BF16, kind="ExternalInput")
    w_qkv = nc.dram_tensor("w_qkv", [d_in, 3 * dh], BF16, kind="ExternalInput")
    b_qkv = nc.dram_tensor("b_qkv", [1, 3 * dh], F32, kind="ExternalInput")
    w_out = nc.dram_tensor("w_out", [dh, d_out], BF16, kind="ExternalInput")
    b_out = nc.dram_tensor("b_out", [1, d_out], F32, kind="ExternalInput")
    tri = nc.dram_tensor("tri", [128, sw], BF16, kind="ExternalInput")
    out = nc.dram_tensor("out", [r, d_out], F32, kind="ExternalOutput")

    with tile.TileContext(nc) as tc:
        with (
            tc.tile_pool(name="dram", bufs=1, space="DRAM") as dram,
            tc.tile_pool(name="consts", bufs=1) as consts,
            tc.tile_pool(name="params", bufs=1) as params,
            tc.tile_pool(name="qkv", bufs=1) as qkvp,
            tc.tile_pool(name="gath", bufs=1) as gath,
            tc.tile_pool(name="accs", bufs=1) as accs,
            tc.tile_pool(name="stage", bufs=2) as stagep,
            tc.tile_pool(name="exps", bufs=6) as exps,
            tc.tile_pool(name="dyn", bufs=6) as dynp,
            tc.tile_pool(name="epi", bufs=2) as epip,
            tc.tile_pool(name="outp", bufs=3) as outpp,
            tc.tile_pool(name="st_ps", bufs=2, space="PSUM") as st_ps,
            tc.tile_pool(name="o1_ps", bufs=1, space="PSUM") as o1_ps,
            tc.tile_pool(name="rs1_ps", bufs=1, space="PSUM") as rs1_ps,
            tc.tile_pool(name="misc_ps", bufs=2, space="PSUM") as misc_ps,
        ):
            from concourse.tile_rust import add_dep_helper

            # ---------------- input loads (spread across queues) ----------
            # wqkv first on the sync queue (gates the first projection),
            # then x half 0 behind it; x half 1 on the vector queue; w_out
            # on the scalar queue (which must stay short for the bounces).
            wqkv_bf = params.tile([128, kd, 3 * dh], BF16, tag="wqkv_bf")
            for k in range(kd):
                nc.sync.dma_start(
                    wqkv_bf[:, k, :], w_qkv[128 * k:128 * (k + 1), :])
            wout_bf = params.tile([dh, d_out], BF16, tag="wout_bf")
            nc.scalar.dma_start(wout_bf[:], w_out[:, :])
            bqkv_bf = params.tile([1, 3 * dh], BF16, tag="bqkv_bf")
            bout_bf = params.tile([1, d_out], BF16, tag="bout_bf")
            if not skip_bias:
                st = stagep.tile([1, 3 * dh], F32, tag="stage_b")
                nc.sync.dma_start(st[:], b_qkv[:, :])
                nc.vector.tensor_copy(bqkv_bf[:], st[:])
                st2 = stagep.tile([1, d_out], F32, tag="stage_b2")
                nc.sync.dma_start(st2[:], b_out[:, :])
                nc.vector.tensor_copy(bout_bf[:], st2[:])
            xT_bf = params.tile([128, kd, r], BF16, tag="xT_bf")
            for k in range(kd):
                nc.sync.dma_start(
                    xT_bf[:, k, 0:ch], xT[128 * k:128 * (k + 1), 0:ch])
            for k in range(kd):
                nc.scalar.dma_start(
                    xT_bf[:, k, ch:2 * ch], xT[128 * k:128 * (k + 1), ch:2 * ch])

            # ---------------- constants (vector engine only) --------------
            ones_col = consts.tile([128, 1], BF16, tag="ones_col")
            nc.vector.memset(ones_col[:], 1.0)
            ones_row = consts.tile([1, max(ch, 128)], BF16, tag="ones_row")
            nc.vector.memset(ones_row[:], 1.0)
            one_f = consts.tile([1, 1], F32, tag="one_f")
            nc.vector.memset(one_f[:], 1.0)
            # warm the exp activation-table set before the first real exp
            warm = consts.tile([1, 1], F32, tag="warm")
            nc.scalar.activation(warm[0:1, 0:1], one_f[0:1, 0:1], AF.Exp)
            # causal triangle masks (host constant), [128, ch] per sub-chunk
            masks = consts.tile([128, sw], BF16, tag="masks")
            nc.sync.dma_start(masks[:], tri[:, :])
            O_acc = accs.tile([128, 2 * ch], F32, tag="O_acc")
            rs_acc = accs.tile([1, 2 * ch], F32, tag="rs_acc")
            nc.vector.memset(O_acc[:], 0.0)
            nc.vector.memset(rs_acc[:], 0.0)

            # ------- per-half: project k/v (fp8), bounce + all-gather -----
            qT_bf = qkvp.tile([128, r], BF16, tag="qT_bf")
            kT_loc = [qkvp.tile([128, ch], BF16, tag=f"kT_loc{h}",
                                 name=f"kT_loc{h}") for h in range(2)]
            v_loc = [qkvp.tile([128, sub, dh], BF16, tag=f"v_loc{h}",
                                name=f"v_loc{h}") for h in range(2)]
            rg = [list(range(cores))]
            cc_insts = []
            last_bounce = None
            for h in range(2):
                # kT half h
                ps = misc_ps.tile([128, ch], F32, tag="mps")
                for k in range(kd):
                    nc.tensor.matmul(
                        ps[:],
                        lhsT=wqkv_bf[:, k, dh:2 * dh],
                        rhs=xT_bf[:, k, h * ch:(h + 1) * ch],
                        start=(k == 0), stop=(skip_bias and k == kd - 1))
                if not skip_bias:
                    nc.tensor.matmul(
                        ps[:], lhsT=bqkv_bf[0:1, dh:2 * dh],
                        rhs=ones_row[0:1, 0:ch], start=False, stop=True)
                # cast to fp8 on the scalar engine (keeps the bounce DMA,
                # also on the scalar queue, free of cross-engine waits)
                nc.scalar.activation(kT_loc[h][:], ps[:], AF.Identity)
                # v tiles of half h
                for t in range(sub):
                    ps = misc_ps.tile([128, dh], F32, tag="mps")
                    for k in range(kd):
                        nc.tensor.matmul(
                            ps[:],
                            lhsT=xT_bf[:, k,
                                       h * ch + 128 * t:h * ch + 128 * (t + 1)],
                            rhs=wqkv_bf[:, k, 2 * dh:3 * dh],
                            start=(k == 0), stop=(skip_bias and k == kd - 1))
                    if not skip_bias:
                        nc.tensor.matmul(
                            ps[:], lhsT=ones_row[0:1, 0:128],
                            rhs=bqkv_bf[0:1, 2 * dh:3 * dh],
                            start=False, stop=True)
                    nc.scalar.activation(v_loc[h][:, t, :], ps[:], AF.Identity)
                # bounce + all-gather half h (fp8 payload, scalar HWDGE)
                kv_b = dram.tile([2 * dh, ch], BF16, tag=f"kv_bounce{h}")
                kv_g = nc.dram_tensor(f"kv_gath{h}", [cores * 2 * dh, ch],
                                      BF16, addr_space="Shared")
                nc.scalar.dma_start(kv_b[0:dh, :], kT_loc[h][:])
                last_bounce = nc.scalar.dma_start(
                    kv_b[dh:2 * dh, :].rearrange("p (t d) -> p t d", t=sub),
                    v_loc[h][:])
                cc = nc.gpsimd.collective_compute(
                    "AllGather", ALU.bypass, replica_groups=rg,
                    ins=[kv_b.opt()], outs=[kv_g.ap().opt()])
                cc_insts.append((cc, kv_g))
            # q^T (after bounces, overlaps the gathers)
            for h in range(2):
                ps = misc_ps.tile([128, ch], F32, tag="mps")
                for k in range(kd):
                    nc.tensor.matmul(
                        ps[:],
                        lhsT=wqkv_bf[:, k, 0:dh],
                        rhs=xT_bf[:, k, h * ch:(h + 1) * ch],
                        start=(k == 0), stop=(skip_bias and k == kd - 1))
                if not skip_bias:
                    nc.tensor.matmul(
                        ps[:], lhsT=bqkv_bf[0:1, 0:dh],
                        rhs=ones_row[0:1, 0:ch], start=False, stop=True)
                nc.vector.tensor_copy(qT_bf[:, h * ch:(h + 1) * ch], ps[:])

            # stage gathered kv into SBUF (cat layout only).
            # cat slot s<8 holds chunk s (gather0 slot s); slot s>=8 holds
            # chunk 23-s (gather1 slot s-8, natural order).
            kT_cat = gath.tile([128, nchunk, ch], BF16, tag="kT_cat")
            v_cat = gath.tile([128, nchunk, sub, dh], BF16, tag="v_cat")
            for hh in range(2):
                cc, kv_g = cc_insts[hh]
                src = kv_g.ap().rearrange("(r t p) c -> t p r c", t=2, p=128)
                d1 = nc.sync.dma_start(
                    kT_cat[:, cores * hh:cores * (hh + 1), :], src[0])
                d2 = nc.sync.dma_start(
                    v_cat[:, cores * hh:cores * (hh + 1), :, :],
                    src[1].rearrange("p r (t d) -> p r t d", t=sub))
                add_dep_helper(d1.ins, cc.ins, sync=True,
                               reason="gather staging waits on collective")
                add_dep_helper(d2.ins, cc.ins, sync=True,
                               reason="gather staging waits on collective")

            # ---------------- attention (software-pipelined) ----------
            # Item k's score matmuls + exps are emitted before item k-1's
            # PV/rowsum matmuls, so the tensor queue never head-of-line
            # blocks on an exp in flight: while the scalar engine computes
            # exp(k), the tensor engine runs the next score matmuls.
            c_reg = nc.vector.partition_id()

            O1 = o1_ps.tile([128, ch], F32, tag="O1")
            rs1 = rs1_ps.tile([1, ch], F32, tag="rs1")

            n_static = cores
            ngroups = sub // gs

            class Item:
                pass

            def emit_st_exp(itm):
                """Score matmuls + exps (+ causal mask) for one item."""
                itm.ex = []
                itm.first_mm = None
                for gi in range(ngroups):
                    g = gi * gs
                    stp = st_ps.tile([128, gs * ch], F32, tag="St")
                    for ui in range(gs):
                        mm = nc.tensor.matmul(
                            stp[:, ui * ch:(ui + 1) * ch],
                            lhsT=itm.k_fn(g + ui), rhs=itm.q_ap,
                            start=True, stop=True)
                        itm.first_mm = itm.first_mm or mm
                    ex = exps.tile([128, gs * ch], BF16, tag="ex")
                    e_i = nc.scalar.activation(ex[:], stp[:], AF.Exp,
                                               scale=scale)
                    if itm.act_after is not None:
                        add_dep_helper(e_i.ins, itm.act_after, sync=False,
                                       reason="exp after bounce dma")
                    if itm.mask:
                        nc.vector.tensor_mul(
                            ex[:], ex[:],
                            masks[:, g * ch:(g + gs) * ch])
                    itm.ex.append(ex)

            def emit_pv_rs(itm):
                """Weighted-value + rowsum matmuls (and SBUF accumulation)."""
                o_start, o_stop = itm.startstop
                if itm.o_ps is None:
                    itm.o_ps = misc_ps.tile([128, ch], F32, tag="mps")
                    itm.rs_ps = misc_ps.tile([1, ch], F32, tag="mps")
                for gi in range(ngroups):
                    g = gi * gs
                    ex = itm.ex[gi]
                    for ui in range(gs):
                        u = g + ui
                        nc.tensor.matmul(
                            itm.o_ps[:],
                            lhsT=itm.v_fn(u),
                            rhs=ex[:, ui * ch:(ui + 1) * ch],
                            start=(o_start and u == 0),
                            stop=(o_stop and u == sub - 1))
                    for ui in range(gs):
                        u = g + ui
                        itm.last_mm = nc.tensor.matmul(
                            itm.rs_ps[0:1, :],
                            lhsT=ones_col[:, 0:1],
                            rhs=ex[:, ui * ch:(ui + 1) * ch],
                            start=(o_start and u == 0),
                            stop=(o_stop and u == sub - 1))
                if itm.acc_sl is not None:
                    sl = itm.acc_sl
                    nc.vector.tensor_add(
                        O_acc[:, sl], O_acc[:, sl], itm.o_ps[:])
                    itm.last_add = nc.vector.tensor_add(
                        rs_acc[0:1, sl], rs_acc[0:1, sl], itm.rs_ps[0:1, :])

            def mk(k_fn, v_fn, q_ap, o_ps, rs_ps, startstop, acc_sl,
                   mask=False, act_after=None, pre=None):
                itm = Item()
                itm.k_fn, itm.v_fn, itm.q_ap = k_fn, v_fn, q_ap
                itm.o_ps, itm.rs_ps = o_ps, rs_ps
                itm.startstop, itm.acc_sl = startstop, acc_sl
                itm.mask, itm.act_after, itm.pre = mask, act_after, pre
                return itm

            items = []
            for h in range(2):
                items.append(mk(
                    lambda u, h=h: kT_loc[h][:, 128 * u:128 * (u + 1)],
                    lambda u, h=h: v_loc[h][:, u, :],
                    qT_bf[:, h * ch:(h + 1) * ch],
                    None, None, (True, True),
                    slice(h * ch, (h + 1) * ch),
                    mask=True,
                    act_after=last_bounce.ins if h == 0 else None))
            for t in range(n_static):
                items.append(mk(
                    lambda u, b=t: kT_cat[:, b, 128 * u:128 * (u + 1)],
                    lambda u, b=t: v_cat[:, b, u, :],
                    qT_bf[:, ch:2 * ch],
                    O1, rs1, (t == 0, t == n_static - 1), None))
            dyn_items = []
            for tq in range(cores - 1):
                itm = mk(None, None, None, None, None, (True, True), None)
                itm.tq = tq
                dyn_items.append(itm)
                items.append(itm)

            def emit_dyn_pre(itm):
                tq = itm.tq
                isl = nc.snap((22 - tq - c_reg) >> 4,
                              donate=True, min_val=0, max_val=1)
                blk = nc.snap(tq + c_reg - (cores - 1)
                              + isl * (nchunk - 1 - c_reg),
                              donate=True, min_val=0, max_val=nchunk - 1)
                g_reg = nc.snap(blk >> 3, donate=True, min_val=0, max_val=1)
                slot = nc.snap(blk + g_reg * ((3 * cores - 1) - 2 * blk),
                               donate=True, min_val=0, max_val=nchunk - 1)
                qst = dynp.tile([128, ch], BF16, tag="qst")
                nc.vector.tensor_copy(qst[:], qT_bf[:, ds(isl * ch, ch)])
                kst = dynp.tile([128, 1, ch], BF16, tag="kst")
                nc.vector.tensor_copy(kst[:], kT_cat[:, ds(slot, 1), :])
                vst = dynp.tile([128, 1, sub, dh], BF16, tag="vst")
                nc.vector.tensor_copy(vst[:], v_cat[:, ds(slot, 1), :, :])
                itm.k_fn = lambda u: kst[:, 0, 128 * u:128 * (u + 1)]
                itm.v_fn = lambda u: vst[:, 0, u, :]
                itm.q_ap = qst[:]
                itm.acc_sl = ds(isl * ch, ch)

            # pipelined emission: st/exp of item k, then pv/rs of item k-1
            prev = None
            for k, itm in enumerate(items):
                if hasattr(itm, "tq"):
                    emit_dyn_pre(itm)
                emit_st_exp(itm)
                if prev is not None:
                    emit_pv_rs(prev)
                prev = itm
            emit_pv_rs(prev)

            # arrival gates (see kernel3 notes): marker nops pinned after the
            # diagonal work; static/dynamic reads ordered behind them.
            mark_t = nc.tensor.nop(nofuse=True, hint="kv_arrival_t")
            mark_v = nc.vector.nop(nofuse=True, hint="kv_arrival_v")
            add_dep_helper(mark_t.ins, items[1].last_mm.ins, sync=False,
                           reason="marker after diag matmuls")
            add_dep_helper(mark_v.ins, items[1].last_add.ins, sync=False,
                           reason="marker after diag adds")
            for t in range(n_static):
                add_dep_helper(items[2 + t].first_mm.ins, mark_t.ins,
                               sync=False, reason="static after gather0")

            # ---------------- epilogue ----------------
            for h in range(2):
                Ot = epip.tile([128, ch], BF16, tag="Ot")
                rs_row = epip.tile([1, ch], F32, tag="rs_row")
                if h == 1:
                    nc.vector.tensor_add(Ot[:], O_acc[:, ch:2 * ch], O1[:])
                    nc.vector.tensor_add(rs_row[:], rs_acc[0:1, ch:2 * ch],
                                         rs1[0:1, :])
                else:
                    nc.vector.tensor_copy(Ot[:], O_acc[:, 0:ch])
                    nc.vector.tensor_copy(rs_row[:], rs_acc[0:1, 0:ch])
                rs_bf = epip.tile([1, ch], BF16, tag="rs_bf")
                if not skip_bias:
                    nc.vector.tensor_copy(rs_bf[:], rs_row[:])
                for tt in range(it):
                    rsT = misc_ps.tile([128, 1], F32, tag="mps")
                    nc.tensor.matmul(
                        rsT[:],
                        lhsT=rs_row[0:1, 128 * tt:128 * (tt + 1)],
                        rhs=one_f[0:1, 0:1], start=True, stop=True)
                    rec = epip.tile([128, 1], F32, tag="rec")
                    nc.vector.reciprocal(rec[:], rsT[:])
                    osb = outpp.tile([128, d_out], F32, tag="osb")
                    for m in range(mh):
                        ops = misc_ps.tile([128, m_t], F32, tag="mps")
                        nc.tensor.matmul(
                            ops[:],
                            lhsT=Ot[:, 128 * tt:128 * (tt + 1)],
                            rhs=wout_bf[:, m * m_t:(m + 1) * m_t],
                            start=True, stop=skip_bias)
                        if not skip_bias:
                            nc.tensor.matmul(
                                ops[:],
                                lhsT=rs_bf[0:1, 128 * tt:128 * (tt + 1)],
                                rhs=bout_bf[0:1, m * m_t:(m + 1) * m_t],
                                start=False, stop=True)
                        nc.vector.tensor_scalar_mul(
                            osb[:, m * m_t:(m + 1) * m_t], ops[:],
                            rec[:, 0:1])
                    dma_eng = nc.sync if tt % 2 == 0 else nc.scalar
                    dma_eng.dma_start(
                        out[h * ch + 128 * tt: h * ch + 128 * (tt + 1), :],
                        osb[:])

    nc.compile()
    return nc


# ---------------- host side ----------------

_CACHED = {}


def _get_program(key, **kw):
    if key not in _CACHED:
        _CACHED[key] = build_program(**kw)
    return _CACHED[key]


def shard_inputs(x, w_qkv, b_qkv, w_out, b_out, cores=8):
    import ml_dtypes
    n = x.shape[0]
    nchunk = 2 * cores
    ch = n // nchunk
    sub = ch // 128
    ii = np.arange(ch)[None, :]
    jj = np.arange(128)[:, None]
    tri = np.concatenate(
        [(ii >= 128 * u + jj) for u in range(sub)],
        axis=1).astype(ml_dtypes.bfloat16)
    wq = np.ascontiguousarray(w_qkv).astype(ml_dtypes.bfloat16)
    wo = np.ascontiguousarray(w_out).astype(ml_dtypes.bfloat16)
    bq = np.ascontiguousarray(b_qkv).reshape(1, -1).astype(np.float32)
    bo = np.ascontiguousarray(b_out).reshape(1, -1).astype(np.float32)
    in_maps = []
    for c in range(cores):
        xs = np.concatenate(
            [x[ch * c: ch * (c + 1)],
             x[ch * (nchunk - 1 - c): ch * (nchunk - c)]], axis=0)
        in_maps.append({
            "xT": np.ascontiguousarray(xs.T).astype(ml_dtypes.bfloat16),
            "w_qkv": wq, "b_qkv": bq, "w_out": wo, "b_out": bo, "tri": tri,
        })
    return in_maps


def unshard_output(results, n, d_out, cores=8):
    nchunk = 2 * cores
    ch = n // nchunk
    out = np.empty((n, d_out), dtype=np.float32)
    for c in range(cores):
        o = results[c]["out"]
        out[ch * c: ch * (c + 1)] = o[:ch]
        out[ch * (nchunk - 1 - c): ch * (nchunk - c)] = o[ch:]
    return out


def kernel(x, w_qkv, b_qkv, w_out, b_out):
    from concourse.bass_utils import run_bass_kernel_spmd

    x = np.asarray(x)
    w_qkv = np.asarray(w_qkv)
    b_qkv = np.asarray(b_qkv)
    w_out = np.asarray(w_out)
    b_out = np.asarray(b_out)
    cores = 8
    n, d_in = x.shape
    d_out = w_out.shape[1]
    dh = w_out.shape[0]
    skip_bias = not (np.any(b_qkv) or np.any(b_out))
    nc = _get_program(
        (cores, n, d_in, d_out, dh, skip_bias),
        cores=cores, n=n, d_in=d_in, d_out=d_out, dh=dh,
        skip_bias=skip_bias)
    in_maps = shard_inputs(x, w_qkv, b_qkv, w_out, b_out, cores)
    res = run_bass_kernel_spmd(nc, in_maps, core_ids=list(range(cores)))
    return unshard_output(res.results, n, d_out, cores)


# revision 15
# speedup vs baseline: 1.0062x; 1.0062x over previous
"""Distributed causal attention kernel for one TRN2 chip (8 NeuronCores).

Problem: out = (softmax_causal((xWq)(xWk)^T / sqrt(dh)) (xWv)) Wout + b
  N=8192, D_IN=1024, D_HEAD=128, D_OUT=1024, fp32 I/O (bf16/fp8 compute).

Sharding (zig-zag for causal load balance): the sequence is split into
16 chunks of 512 rows; core c owns chunks c and 15-c, so every core has
the same causal attention area (17 blocks of 512x512).  Q stays local,
K/V shards are computed locally and AllGather'ed (bf16).

Layout: scores are computed transposed, St[j, i] = K Q^T, so that the
softmax-weighted PV matmul needs no transposes: O^T[dh, i] = V^T P^T via
lhsT = V (natural), rhs = exp(St).  Softmax skips the max-subtraction
(scores are ~N(0,1), |s| < ~7) and defers normalization: the row-sum is
accumulated with a ones-vector matmul and the division happens after
the output projection.

Scheduling: the gpsimd queue carries ONLY the collective instructions,
so the NRT entry barrier starts as early as possible and overlaps the
projections.  Input loads are spread across the sync/scalar DMA queues
in per-chunk pieces; weights arrive pre-cast to bf16 and the causal
mask is a host constant, so no staging copies block the start.  The
attention items are software-pipelined (item k's score matmuls + exps
are emitted before item k-1's PV/rowsum) so the tensor queue never
head-of-line blocks on an exp in flight.  The scalar engine does only
exps; the epilogue scaling runs on the vector engine.

SPMD uniformity: all cores run one program.  Of the 17 causal work
items per core, 2 are the diagonal blocks (local k/v, computed while
the all-gather is in flight), 8 are statically identical across cores,
and 7 select their (q-half, kv-block) via DVE registers derived from
partition_id and dynamic `ds()` slices, with PV partials accumulated
into an SBUF accumulator by the vector engine.
"""

import sys

import numpy as np

if "/opt/trn_rl_repo" not in sys.path:
    sys.path.insert(0, "/opt/trn_rl_repo")

import concourse.mybir as mybir
import concourse.tile as tile
from concourse import bacc
from concourse.bass import ds

F32 = mybir.dt.float32
BF16 = mybir.dt.bfloat16
F8 = mybir.dt.float8e4
AF = mybir.ActivationFunctionType
ALU = mybir.AluOpType
DR = mybir.MatmulPerfMode.DoubleRow
EXP_BIAS = -1.5


def build_program(cores=8, n=8192, d_in=1024, d_out=1024, dh=128,
                  enable_asserts=False, skip_bias=False):
    nchunk = 2 * cores            # zig-zag chunks
    ch = n // nchunk              # rows per chunk (512)
    r = 2 * ch                    # rows per core (1024)
    kd = d_in // 128              # contraction chunks for projections
    sub = ch // 128               # 128-row sub-chunks per kv block
    it = ch // 128                # 128-row i-tiles per half
    scale = float(dh) ** -0.5
    sw = sub * ch                 # score tile width (free elems per item)
    m_t = 512 if d_out >= 512 else d_out   # out-proj moving width
    mh = d_out // m_t
    gs = max(1, sub // 2)         # subchunks per St group (double-buffer)

    nc = bacc.Bacc("TRN2", target_bir_lowering=False, debug=False,
                   num_devices=cores, enable_asserts=enable_asserts)

    xT = nc.dram_tensor("xT", [d_in, r], BF16, kind="ExternalInput")
    w_qkv = nc.dram_tensor("w_qkv", [d_in, 3 * dh], BF16, kind="ExternalInput")
    b_qkv = nc.dram_tensor("b_qkv", [1, 3 * dh], F32, kind="ExternalInput")
    w_out = nc.dram_tensor("w_out", [dh, d_out], BF16, kind="ExternalInput")
    b_out = nc.dram_tensor("b_out", [1, d_out], F32, kind="ExternalInput")
    tri = nc.dram_tensor("tri", [128, sw], BF16, kind="ExternalInput")
    out = nc.dram_tensor("out", [r, d_out], F32, kind="ExternalOutput")

    with tile.TileContext(nc) as tc:
        with (
            tc.tile_pool(name="dram", bufs=1, space="DRAM") as dram,
            tc.tile_pool(name="consts", bufs=1) as consts,
            tc.tile_pool(name="params", bufs=1) as params,
            tc.tile_pool(name="qkv", bufs=1) as qkvp,
            tc.tile_pool(name="gath", bufs=1) as gath,
            tc.tile_pool(name="accs", bufs=1) as accs,
            tc.tile_pool(name="stage", bufs=2) as stagep,
            tc.tile_pool(name="exps", bufs=6) as exps,
            tc.tile_pool(name="dyn", bufs=6) as dynp,
            tc.tile_pool(name="epi", bufs=2) as epip,
            tc.tile_pool(name="outp", bufs=3) as outpp,
            tc.tile_pool(name="st_ps", bufs=2, space="PSUM") as st_ps,
            tc.tile_pool(name="o1_ps", bufs=1, space="PSUM") as o1_ps,
            tc.tile_pool(name="rs1_ps", bufs=1, space="PSUM") as rs1_ps,
            tc.tile_pool(name="misc_ps", bufs=2, space="PSUM") as misc_ps,
        ):
            from concourse.tile_rust import add_dep_helper

            # ---------------- input loads (spread across queues) ----------
            # wqkv first on the sync queue (gates the first projection),
            # then x half 0 behind it; x half 1 on the vector queue; w_out
            # on the scalar queue (which must stay short for the bounces).
            wqkv_bf = params.tile([128, kd, 3 * dh], BF16, tag="wqkv_bf")
            for k in range(kd):
                nc.sync.dma_start(
                    wqkv_bf[:, k, :], w_qkv[128 * k:128 * (k + 1), :])
            wout_bf = params.tile([dh, d_out], BF16, tag="wout_bf")
            nc.scalar.dma_start(wout_bf[:], w_out[:, :])
            bqkv_bf = params.tile([1, 3 * dh], BF16, tag="bqkv_bf")
            bout_bf = params.tile([1, d_out], BF16, tag="bout_bf")
            if not skip_bias:
                st = stagep.tile([1, 3 * dh], F32, tag="stage_b")
                nc.sync.dma_start(st[:], b_qkv[:, :])
                nc.vector.tensor_copy(bqkv_bf[:], st[:])
                st2 = stagep.tile([1, d_out], F32, tag="stage_b2")
                nc.sync.dma_start(st2[:], b_out[:, :])
                nc.vector.tensor_copy(bout_bf[:], st2[:])
            xT_bf = params.tile([128, kd, r], BF16, tag="xT_bf")
            for k in range(kd):
                nc.sync.dma_start(
                    xT_bf[:, k, 0:ch], xT[128 * k:128 * (k + 1), 0:ch])
            for k in range(kd):
                nc.scalar.dma_start(
                    xT_bf[:, k, ch:2 * ch], xT[128 * k:128 * (k + 1), ch:2 * ch])

            # ---------------- constants (vector engine only) --------------
            ones_col = consts.tile([128, 1], BF16, tag="ones_col")
            nc.vector.memset(ones_col[:], 1.0)
            ones_row = consts.tile([1, max(ch, 128)], BF16, tag="ones_row")
            nc.vector.memset(ones_row[:], 1.0)
            one_f = consts.tile([1, 1], F32, tag="one_f")
            nc.vector.memset(one_f[:], 1.0)
            # warm the exp activation-table set before the first real exp
            warm = consts.tile([1, 1], F32, tag="warm")
            nc.scalar.activation(warm[0:1, 0:1], one_f[0:1, 0:1], AF.Exp)
            # causal triangle masks (host constant), [128, ch] per sub-chunk
            masks = consts.tile([128, sw], BF16, tag="masks")
            nc.sync.dma_start(masks[:], tri[:, :])
            O_acc = accs.tile([128, 2 * ch], F32, tag="O_acc")
            rs_acc = accs.tile([1, 2 * ch], F32, tag="rs_acc")
            nc.vector.memset(O_acc[:], 0.0)
            nc.vector.memset(rs_acc[:], 0.0)

            # ------- per-half: project k/v (fp8), bounce + all-gather -----
            qT_bf = qkvp.tile([128, r], BF16, tag="qT_bf")
            kT_loc = [qkvp.tile([128, ch], BF16, tag=f"kT_loc{h}",
                                 name=f"kT_loc{h}") for h in range(2)]
            v_loc = [qkvp.tile([128, sub, dh], BF16, tag=f"v_loc{h}",
                                name=f"v_loc{h}") for h in range(2)]
            rg = [list(range(cores))]
            cc_insts = []
            last_bounce = None
            for h in range(2):
                # kT half h
                ps = misc_ps.tile([128, ch], F32, tag="mps")
                for k in range(kd):
                    nc.tensor.matmul(
                        ps[:],
                        lhsT=wqkv_bf[:, k, dh:2 * dh],
                        rhs=xT_bf[:, k, h * ch:(h + 1) * ch],
                        start=(k == 0), stop=(skip_bias and k == kd - 1))
                if not skip_bias:
                    nc.tensor.matmul(
                        ps[:], lhsT=bqkv_bf[0:1, dh:2 * dh],
                        rhs=ones_row[0:1, 0:ch], start=False, stop=True)
                # cast to fp8 on the scalar engine (keeps the bounce DMA,
                # also on the scalar queue, free of cross-engine waits)
                nc.scalar.activation(kT_loc[h][:], ps[:], AF.Identity)
                # v tiles of half h
                for t in range(sub):
                    ps = misc_ps.tile([128, dh], F32, tag="mps")
                    for k in range(kd):
                        nc.tensor.matmul(
                            ps[:],
                            lhsT=xT_bf[:, k,
                                       h * ch + 128 * t:h * ch + 128 * (t + 1)],
                            rhs=wqkv_bf[:, k, 2 * dh:3 * dh],
                            start=(k == 0), stop=(skip_bias and k == kd - 1))
                    if not skip_bias:
                        nc.tensor.matmul(
                            ps[:], lhsT=ones_row[0:1, 0:128],
                            rhs=bqkv_bf[0:1, 2 * dh:3 * dh],
                            start=False, stop=True)
                    nc.scalar.activation(v_loc[h][:, t, :], ps[:], AF.Identity)
                # bounce + all-gather half h (fp8 payload, scalar HWDGE)
                kv_b = dram.tile([2 * dh, ch], BF16, tag=f"kv_bounce{h}")
                kv_g = nc.dram_tensor(f"kv_gath{h}", [cores * 2 * dh, ch],
                                      BF16, addr_space="Shared")
                nc.scalar.dma_start(kv_b[0:dh, :], kT_loc[h][:])
                last_bounce = nc.scalar.dma_start(
                    kv_b[dh:2 * dh, :].rearrange("p (t d) -> p t d", t=sub),
                    v_loc[h][:])
                cc = nc.gpsimd.collective_compute(
                    "AllGather", ALU.bypass, replica_groups=rg,
                    ins=[kv_b.opt()], outs=[kv_g.ap().opt()])
                cc_insts.append((cc, kv_g))
            # q^T (after bounces, overlaps the gathers)
            for h in range(2):
                ps = misc_ps.tile([128, ch], F32, tag="mps")
                for k in range(kd):
                    nc.tensor.matmul(
                        ps[:],
                        lhsT=wqkv_bf[:, k, 0:dh],
                        rhs=xT_bf[:, k, h * ch:(h + 1) * ch],
                        start=(k == 0), stop=(skip_bias and k == kd - 1))
                if not skip_bias:
                    nc.tensor.matmul(
                        ps[:], lhsT=bqkv_bf[0:1, 0:dh],
                        rhs=ones_row[0:1, 0:ch], start=False, stop=True)
                nc.vector.tensor_copy(qT_bf[:, h * ch:(h + 1) * ch], ps[:])

            # stage gathered kv into SBUF (cat layout only).
            # cat slot s<8 holds chunk s (gather0 slot s); slot s>=8 holds
            # chunk 23-s (gather1 slot s-8, natural order).
            kT_cat = gath.tile([128, nchunk, ch], BF16, tag="kT_cat")
            v_cat = gath.tile([128, nchunk, sub, dh], BF16, tag="v_cat")
            for hh in range(2):
                cc, kv_g = cc_insts[hh]
                src = kv_g.ap().rearrange("(r t p) c -> t p r c", t=2, p=128)
                d1 = nc.sync.dma_start(
                    kT_cat[:, cores * hh:cores * (hh + 1), :], src[0])
                d2 = nc.sync.dma_start(
                    v_cat[:, cores * hh:cores * (hh + 1), :, :],
                    src[1].rearrange("p r (t d) -> p r t d", t=sub))
                add_dep_helper(d1.ins, cc.ins, sync=True,
                               reason="gather staging waits on collective")
                add_dep_helper(d2.ins, cc.ins, sync=True,
                               reason="gather staging waits on collective")

            # ---------------- attention (software-pipelined) ----------
            # Item k's score matmuls + exps are emitted before item k-1's
            # PV/rowsum matmuls, so the tensor queue never head-of-line
            # blocks on an exp in flight: while the scalar engine computes
            # exp(k), the tensor engine runs the next score matmuls.
            c_reg = nc.vector.partition_id()

            O1 = o1_ps.tile([128, ch], F32, tag="O1")
            rs1 = rs1_ps.tile([1, ch], F32, tag="rs1")

            n_static = cores
            ngroups = sub // gs

            class Item:
                pass

            def emit_st_exp(itm):
                """Score matmuls + exps (+ causal mask) for one item."""
                itm.ex = []
                itm.first_mm = None
                for gi in range(ngroups):
                    g = gi * gs
                    stp = st_ps.tile([128, gs * ch], F32, tag="St")
                    for ui in range(gs):
                        mm = nc.tensor.matmul(
                            stp[:, ui * ch:(ui + 1) * ch],
                            lhsT=itm.k_fn(g + ui), rhs=itm.q_ap,
                            start=True, stop=True)
                        itm.first_mm = itm.first_mm or mm
                    ex = exps.tile([128, gs * ch], BF16, tag="ex")
                    e_i = nc.scalar.activation(ex[:], stp[:], AF.Exp,
                                               scale=scale)
                    if itm.act_after is not None:
                        add_dep_helper(e_i.ins, itm.act_after, sync=False,
                                       reason="exp after bounce dma")
                    if itm.mask:
                        nc.vector.tensor_mul(
                            ex[:], ex[:],
                            masks[:, g * ch:(g + gs) * ch])
                    itm.ex.append(ex)

            def emit_pv_rs(itm):
                """Weighted-value + rowsum matmuls (and SBUF accumulation)."""
                o_start, o_stop = itm.startstop
                if itm.o_ps is None:
                    itm.o_ps = misc_ps.tile([128, ch], F32, tag="mps")
                    itm.rs_ps = misc_ps.tile([1, ch], F32, tag="mps")
                for gi in range(ngroups):
                    g = gi * gs
                    ex = itm.ex[gi]
                    for ui in range(gs):
                        u = g + ui
                        nc.tensor.matmul(
                            itm.o_ps[:],
                            lhsT=itm.v_fn(u),
                            rhs=ex[:, ui * ch:(ui + 1) * ch],
                            start=(o_start and u == 0),
                            stop=(o_stop and u == sub - 1))
                    for ui in range(gs):
                        u = g + ui
                        itm.last_mm = nc.tensor.matmul(
                            itm.rs_ps[0:1, :],
                            lhsT=ones_col[:, 0:1],
                            rhs=ex[:, ui * ch:(ui + 1) * ch],
                            start=(o_start and u == 0),
                            stop=(o_stop and u == sub - 1))
                if itm.acc_sl is not None:
                    sl = itm.acc_sl
                    nc.vector.tensor_add(
                        O_acc[:, sl], O_acc[:, sl], itm.o_ps[:])
                    itm.last_add = nc.vector.tensor_add(
                        rs_acc[0:1, sl], rs_acc[0:1, sl], itm.rs_ps[0:1, :])

            def mk(k_fn, v_fn, q_ap, o_ps, rs_ps, startstop, acc_sl,
                   mask=False, act_after=None, pre=None):
                itm = Item()
                itm.k_fn, itm.v_fn, itm.q_ap = k_fn, v_fn, q_ap
                itm.o_ps, itm.rs_ps = o_ps, rs_ps
                itm.startstop, itm.acc_sl = startstop, acc_sl
                itm.mask, itm.act_after, itm.pre = mask, act_after, pre
                return itm

            items = []
            for h in range(2):
                items.append(mk(
                    lambda u, h=h: kT_loc[h][:, 128 * u:128 * (u + 1)],
                    lambda u, h=h: v_loc[h][:, u, :],
                    qT_bf[:, h * ch:(h + 1) * ch],
                    None, None, (True, True),
                    slice(h * ch, (h + 1) * ch),
                    mask=True,
                    act_after=last_bounce.ins if h == 0 else None))
            for t in range(n_static):
                items.append(mk(
                    lambda u, b=t: kT_cat[:, b, 128 * u:128 * (u + 1)],
                    lambda u, b=t: v_cat[:, b, u, :],
                    qT_bf[:, ch:2 * ch],
                    O1, rs1, (t == 0, t == n_static - 1), None))
            dyn_items = []
            for tq in range(cores - 1):
                itm = mk(None, None, None, None, None, (True, True), None)
                itm.tq = tq
                dyn_items.append(itm)
                items.append(itm)

            def emit_dyn_pre(itm):
                tq = itm.tq
                isl = nc.snap((22 - tq - c_reg) >> 4,
                              donate=True, min_val=0, max_val=1)
                blk = nc.snap(tq + c_reg - (cores - 1)
                              + isl * (nchunk - 1 - c_reg),
                              donate=True, min_val=0, max_val=nchunk - 1)
                g_reg = nc.snap(blk >> 3, donate=True, min_val=0, max_val=1)
                slot = nc.snap(blk + g_reg * ((3 * cores - 1) - 2 * blk),
                               donate=True, min_val=0, max_val=nchunk - 1)
                qst = dynp.tile([128, ch], BF16, tag="qst")
                nc.vector.tensor_copy(qst[:], qT_bf[:, ds(isl * ch, ch)])
                kst = dynp.tile([128, 1, ch], BF16, tag="kst")
                nc.vector.tensor_copy(kst[:], kT_cat[:, ds(slot, 1), :])
                vst = dynp.tile([128, 1, sub, dh], BF16, tag="vst")
                nc.vector.tensor_copy(vst[:], v_cat[:, ds(slot, 1), :, :])
                itm.k_fn = lambda u: kst[:, 0, 128 * u:128 * (u + 1)]
                itm.v_fn = lambda u: vst[:, 0, u, :]
                itm.q_ap = qst[:]
                itm.acc_sl = ds(isl * ch, ch)

            # pipelined emission: st/exp of item k, then pv/rs of item k-1
            prev = None
            for k, itm in enumerate(items):
                if hasattr(itm, "tq"):
                    emit_dyn_pre(itm)
                emit_st_exp(itm)
                if prev is not None:
                    emit_pv_rs(prev)
                prev = itm
            emit_pv_rs(prev)

            # arrival gates (see kernel3 notes): marker nops pinned after the
            # diagonal work; static/dynamic reads ordered behind them.
            mark_t = nc.tensor.nop(nofuse=True, hint="kv_arrival_t")
            mark_v = nc.vector.nop(nofuse=True, hint="kv_arrival_v")
            add_dep_helper(mark_t.ins, items[1].last_mm.ins, sync=False,
                           reason="marker after diag matmuls")
            add_dep_helper(mark_v.ins, items[1].last_add.ins, sync=False,
                           reason="marker after diag adds")
            for t in range(n_static):
                add_dep_helper(items[2 + t].first_mm.ins, mark_t.ins,
                               sync=False, reason="static after gather0")

            # ---------------- epilogue ----------------
            for h in range(2):
                Ot = epip.tile([128, ch], BF16, tag="Ot")
                rs_row = epip.tile([1, ch], F32, tag="rs_row")
                if h == 1:
                    nc.vector.tensor_add(Ot[:], O_acc[:, ch:2 * ch], O1[:])
                    nc.vector.tensor_add(rs_row[:], rs_acc[0:1, ch:2 * ch],
                                         rs1[0:1, :])
                else:
                    nc.vector.tensor_copy(Ot[:], O_acc[:, 0:ch])
                    nc.vector.tensor_copy(rs_row[:], rs_acc[0:1, 0:ch])
                rs_bf = epip.tile([1, ch], BF16, tag="rs_bf")
                if not skip_bias:
                    nc.vector.tensor_copy(rs_bf[:], rs_row[:])
                for tt in range(it):
                    rsT = misc_ps.tile([128, 1], F32, tag="mps")
                    nc.tensor.matmul(
                        rsT[:],
                        lhsT=rs_row[0:1, 128 * tt:128 * (tt + 1)],
                        rhs=one_f[0:1, 0:1], start=True, stop=True)
                    rec = epip.tile([128, 1], F32, tag="rec")
                    nc.vector.reciprocal(rec[:], rsT[:])
                    osb = outpp.tile([128, d_out], F32, tag="osb")
                    for m in range(mh):
                        ops = misc_ps.tile([128, m_t], F32, tag="mps")
                        nc.tensor.matmul(
                            ops[:],
                            lhsT=Ot[:, 128 * tt:128 * (tt + 1)],
                            rhs=wout_bf[:, m * m_t:(m + 1) * m_t],
                            start=True, stop=skip_bias)
                        if not skip_bias:
                            nc.tensor.matmul(
                                ops[:],
                                lhsT=rs_bf[0:1, 128 * tt:128 * (tt + 1)],
                                rhs=bout_bf[0:1, m * m_t:(m + 1) * m_t],
                                start=False, stop=True)
                        if (tt * mh + m) % 2 == 0:
                            nc.scalar.activation(
                                osb[:, m * m_t:(m + 1) * m_t], ops[:],
                                AF.Identity, scale=rec[:, 0:1])
                        else:
                            nc.vector.tensor_scalar_mul(
                                osb[:, m * m_t:(m + 1) * m_t], ops[:],
                                rec[:, 0:1])
                    dma_eng = nc.sync if tt % 2 == 0 else nc.scalar
                    dma_eng.dma_start(
                        out[h * ch + 128 * tt: h * ch + 128 * (tt + 1), :],
                        osb[:])

    nc.compile()
    return nc


# ---------------- host side ----------------

_CACHED = {}


def _get_program(key, **kw):
    if key not in _CACHED:
        _CACHED[key] = build_program(**kw)
    return _CACHED[key]


def shard_inputs(x, w_qkv, b_qkv, w_out, b_out, cores=8):
    import ml_dtypes
    n = x.shape[0]
    nchunk = 2 * cores
    ch = n // nchunk
    sub = ch // 128
    ii = np.arange(ch)[None, :]
    jj = np.arange(128)[:, None]
    tri = np.concatenate(
        [(ii >= 128 * u + jj) for u in range(sub)],
        axis=1).astype(ml_dtypes.bfloat16)
    wq = np.ascontiguousarray(w_qkv).astype(ml_dtypes.bfloat16)
    wo = np.ascontiguousarray(w_out).astype(ml_dtypes.bfloat16)
    bq = np.ascontiguousarray(b_qkv).reshape(1, -1).astype(np.float32)
    bo = np.ascontiguousarray(b_out).reshape(1, -1).astype(np.float32)
    in_maps = []
    for c in range(cores):
        xs = np.concatenate(
            [x[ch * c: ch * (c + 1)],
             x[ch * (nchunk - 1 - c): ch * (nchunk - c)]], axis=0)
        in_maps.append({
            "xT": np.ascontiguousarray(xs.T).astype(ml_dtypes.bfloat16),
            "w_qkv": wq, "b_qkv": bq, "w_out": wo, "b_out": bo, "tri": tri,
        })
    return in_maps


def unshard_output(results, n, d_out, cores=8):
    nchunk = 2 * cores
    ch = n // nchunk
    out = np.empty((n, d_out), dtype=np.float32)
    for c in range(cores):
        o = results[c]["out"]
        out[ch * c: ch * (c + 1)] = o[:ch]
        out[ch * (nchunk - 1 - c): ch * (nchunk - c)] = o[ch:]
    return out


def kernel(x, w_qkv, b_qkv, w_out, b_out):
    from concourse.bass_utils import run_bass_kernel_spmd

    x = np.asarray(x)
    w_qkv = np.asarray(w_qkv)
    b_qkv = np.asarray(b_qkv)
    w_out = np.asarray(w_out)
    b_out = np.asarray(b_out)
    cores = 8
    n, d_in = x.shape
    d_out = w_out.shape[1]
    dh = w_out.shape[0]
    skip_bias = not (np.any(b_qkv) or np.any(b_out))
    nc = _get_program(
        (cores, n, d_in, d_out, dh, skip_bias),
        cores=cores, n=n, d_in=d_in, d_out=d_out, dh=dh,
        skip_bias=skip_bias)
    in_maps = shard_inputs(x, w_qkv, b_qkv, w_out, b_out, cores)
    res = run_bass_kernel_spmd(nc, in_maps, core_ids=list(range(cores)))
    return unshard_output(res.results, n, d_out, cores)


# revision 16
# speedup vs baseline: 1.1838x; 1.1765x over previous
"""Distributed causal attention kernel for one TRN2 chip (8 NeuronCores).

Problem: out = (softmax_causal((xWq)(xWk)^T / sqrt(dh)) (xWv)) Wout + b
  N=8192, D_IN=1024, D_HEAD=128, D_OUT=1024, fp32 I/O (bf16/fp8 compute).

Sharding (zig-zag for causal load balance): the sequence is split into
16 chunks of 512 rows; core c owns chunks c and 15-c, so every core has
the same causal attention area (17 blocks of 512x512).  Q stays local,
K/V shards are computed locally and AllGather'ed (bf16).

Layout: scores are computed transposed, St[j, i] = K Q^T, so that the
softmax-weighted PV matmul needs no transposes: O^T[dh, i] = V^T P^T via
lhsT = V (natural), rhs = exp(St).  Softmax skips the max-subtraction
(scores are ~N(0,1), |s| < ~7) and defers normalization: the row-sum is
accumulated with a ones-vector matmul and the division happens after
the output projection.

Scheduling: the gpsimd queue carries ONLY the collective instructions,
so the NRT entry barrier starts as early as possible and overlaps the
projections.  Input loads are spread across the sync/scalar DMA queues
in per-chunk pieces; weights arrive pre-cast to bf16 and the causal
mask is a host constant, so no staging copies block the start.  The
attention items are software-pipelined (item k's score matmuls + exps
are emitted before item k-1's PV/rowsum) so the tensor queue never
head-of-line blocks on an exp in flight.  The scalar engine does only
exps; the epilogue scaling runs on the vector engine.

SPMD uniformity: all cores run one program.  Of the 17 causal work
items per core, 2 are the diagonal blocks (local k/v, computed while
the all-gather is in flight), 8 are statically identical across cores,
and 7 select their (q-half, kv-block) via DVE registers derived from
partition_id and dynamic `ds()` slices, with PV partials accumulated
into an SBUF accumulator by the vector engine.
"""

import sys

import numpy as np

if "/opt/trn_rl_repo" not in sys.path:
    sys.path.insert(0, "/opt/trn_rl_repo")

import concourse.mybir as mybir
import concourse.tile as tile
from concourse import bacc
from concourse.bass import ds

F32 = mybir.dt.float32
BF16 = mybir.dt.bfloat16
F8 = mybir.dt.float8e4
AF = mybir.ActivationFunctionType
ALU = mybir.AluOpType
DR = mybir.MatmulPerfMode.DoubleRow
EXP_BIAS = -1.5


def build_program(cores=8, n=8192, d_in=1024, d_out=1024, dh=128,
                  enable_asserts=False, skip_bias=False):
    nchunk = 2 * cores            # zig-zag chunks
    ch = n // nchunk              # rows per chunk (512)
    r = 2 * ch                    # rows per core (1024)
    kd = d_in // 128              # contraction chunks for projections
    sub = ch // 128               # 128-row sub-chunks per kv block
    it = ch // 128                # 128-row i-tiles per half
    scale = float(dh) ** -0.5
    sw = sub * ch                 # score tile width (free elems per item)
    m_t = 512 if d_out >= 512 else d_out   # out-proj moving width
    mh = d_out // m_t
    gs = max(1, sub // 2)         # subchunks per St group (double-buffer)

    nc = bacc.Bacc("TRN2", target_bir_lowering=False, debug=False,
                   num_devices=cores, enable_asserts=enable_asserts)

    xT = nc.dram_tensor("xT", [d_in, r], BF16, kind="ExternalInput")
    w_qkv = nc.dram_tensor("w_qkv", [d_in, 3 * dh], BF16, kind="ExternalInput")
    b_qkv = nc.dram_tensor("b_qkv", [1, 3 * dh], F32, kind="ExternalInput")
    w_out = nc.dram_tensor("w_out", [dh, d_out], BF16, kind="ExternalInput")
    b_out = nc.dram_tensor("b_out", [1, d_out], F32, kind="ExternalInput")
    tri = nc.dram_tensor("tri", [128, sw], BF16, kind="ExternalInput")
    out = nc.dram_tensor("out", [r, d_out], F32, kind="ExternalOutput")

    with tile.TileContext(nc) as tc:
        with (
            tc.tile_pool(name="dram", bufs=1, space="DRAM") as dram,
            tc.tile_pool(name="consts", bufs=1) as consts,
            tc.tile_pool(name="params", bufs=1) as params,
            tc.tile_pool(name="qkv", bufs=1) as qkvp,
            tc.tile_pool(name="gath", bufs=1) as gath,
            tc.tile_pool(name="accs", bufs=1) as accs,
            tc.tile_pool(name="stage", bufs=2) as stagep,
            tc.tile_pool(name="exps", bufs=6) as exps,
            tc.tile_pool(name="dyn", bufs=6) as dynp,
            tc.tile_pool(name="epi", bufs=2) as epip,
            tc.tile_pool(name="outp", bufs=3) as outpp,
            tc.tile_pool(name="st_ps", bufs=2, space="PSUM") as st_ps,
            tc.tile_pool(name="o1_ps", bufs=1, space="PSUM") as o1_ps,
            tc.tile_pool(name="rs1_ps", bufs=1, space="PSUM") as rs1_ps,
            tc.tile_pool(name="misc_ps", bufs=2, space="PSUM") as misc_ps,
        ):
            from concourse.tile_rust import add_dep_helper

            # ---------------- input loads (spread across queues) ----------
            # wqkv first on the sync queue (gates the first projection),
            # then x half 0 behind it; x half 1 on the vector queue; w_out
            # on the scalar queue (which must stay short for the bounces).
            wqkv_bf = params.tile([128, kd, 3 * dh], BF16, tag="wqkv_bf")
            for k in range(kd):
                nc.sync.dma_start(
                    wqkv_bf[:, k, :], w_qkv[128 * k:128 * (k + 1), :])
            wout_bf = params.tile([dh, d_out], BF16, tag="wout_bf")
            nc.scalar.dma_start(wout_bf[:], w_out[:, :])
            bqkv_bf = params.tile([1, 3 * dh], BF16, tag="bqkv_bf")
            bout_bf = params.tile([1, d_out], BF16, tag="bout_bf")
            if not skip_bias:
                st = stagep.tile([1, 3 * dh], F32, tag="stage_b")
                nc.sync.dma_start(st[:], b_qkv[:, :])
                nc.vector.tensor_copy(bqkv_bf[:], st[:])
                st2 = stagep.tile([1, d_out], F32, tag="stage_b2")
                nc.sync.dma_start(st2[:], b_out[:, :])
                nc.vector.tensor_copy(bout_bf[:], st2[:])
            xT_bf = params.tile([128, kd, r], BF16, tag="xT_bf")
            for k in range(kd):
                nc.sync.dma_start(
                    xT_bf[:, k, 0:ch], xT[128 * k:128 * (k + 1), 0:ch])
            for k in range(kd):
                nc.scalar.dma_start(
                    xT_bf[:, k, ch:2 * ch], xT[128 * k:128 * (k + 1), ch:2 * ch])

            # ---------------- constants (vector engine only) --------------
            ones_col = consts.tile([128, 1], BF16, tag="ones_col")
            nc.vector.memset(ones_col[:], 1.0)
            ones_row = consts.tile([1, max(ch, 128)], BF16, tag="ones_row")
            nc.vector.memset(ones_row[:], 1.0)
            one_f = consts.tile([1, 1], F32, tag="one_f")
            nc.vector.memset(one_f[:], 1.0)
            # warm the exp activation-table set before the first real exp
            warm = consts.tile([1, 1], F32, tag="warm")
            nc.scalar.activation(warm[0:1, 0:1], one_f[0:1, 0:1], AF.Exp)
            # causal triangle masks (host constant), [128, ch] per sub-chunk
            masks = consts.tile([128, sw], BF16, tag="masks")
            nc.sync.dma_start(masks[:], tri[:, :])
            O_acc = accs.tile([128, 2 * ch], F32, tag="O_acc")
            rs_acc = accs.tile([1, 2 * ch], F32, tag="rs_acc")
            nc.vector.memset(O_acc[:], 0.0)
            nc.vector.memset(rs_acc[:], 0.0)

            # ------- per-half: project k/v (fp8), bounce + all-gather -----
            qT_bf = qkvp.tile([128, r], BF16, tag="qT_bf")
            kT_loc = [qkvp.tile([128, ch], BF16, tag=f"kT_loc{h}",
                                 name=f"kT_loc{h}") for h in range(2)]
            v_loc = [qkvp.tile([128, sub, dh], BF16, tag=f"v_loc{h}",
                                name=f"v_loc{h}") for h in range(2)]
            rg = [list(range(cores))]
            cc_insts = []
            last_bounce = None
            for h in range(2):
                # kT half h
                ps = misc_ps.tile([128, ch], F32, tag="mps")
                for k in range(kd):
                    nc.tensor.matmul(
                        ps[:],
                        lhsT=wqkv_bf[:, k, dh:2 * dh],
                        rhs=xT_bf[:, k, h * ch:(h + 1) * ch],
                        start=(k == 0), stop=(skip_bias and k == kd - 1))
                if not skip_bias:
                    nc.tensor.matmul(
                        ps[:], lhsT=bqkv_bf[0:1, dh:2 * dh],
                        rhs=ones_row[0:1, 0:ch], start=False, stop=True)
                # cast to fp8 on the scalar engine (keeps the bounce DMA,
                # also on the scalar queue, free of cross-engine waits)
                nc.scalar.activation(kT_loc[h][:], ps[:], AF.Identity)
                # v tiles of half h
                for t in range(sub):
                    ps = misc_ps.tile([128, dh], F32, tag="mps")
                    for k in range(kd):
                        nc.tensor.matmul(
                            ps[:],
                            lhsT=xT_bf[:, k,
                                       h * ch + 128 * t:h * ch + 128 * (t + 1)],
                            rhs=wqkv_bf[:, k, 2 * dh:3 * dh],
                            start=(k == 0), stop=(skip_bias and k == kd - 1))
                    if not skip_bias:
                        nc.tensor.matmul(
                            ps[:], lhsT=ones_row[0:1, 0:128],
                            rhs=bqkv_bf[0:1, 2 * dh:3 * dh],
                            start=False, stop=True)
                    nc.scalar.activation(v_loc[h][:, t, :], ps[:], AF.Identity)
                # bounce + all-gather half h (fp8 payload, scalar HWDGE)
                kv_b = dram.tile([2 * dh, ch], BF16, tag=f"kv_bounce{h}")
                kv_g = nc.dram_tensor(f"kv_gath{h}", [cores * 2 * dh, ch],
                                      BF16, addr_space="Shared")
                nc.scalar.dma_start(kv_b[0:dh, :], kT_loc[h][:])
                last_bounce = nc.scalar.dma_start(
                    kv_b[dh:2 * dh, :].rearrange("p (t d) -> p t d", t=sub),
                    v_loc[h][:])
                cc = nc.gpsimd.collective_compute(
                    "AllGather", ALU.bypass, replica_groups=rg,
                    ins=[kv_b.opt()], outs=[kv_g.ap().opt()])
                cc_insts.append((cc, kv_g))
            # q^T (after bounces, overlaps the gathers)
            for h in range(2):
                ps = misc_ps.tile([128, ch], F32, tag="mps")
                for k in range(kd):
                    nc.tensor.matmul(
                        ps[:],
                        lhsT=wqkv_bf[:, k, 0:dh],
                        rhs=xT_bf[:, k, h * ch:(h + 1) * ch],
                        start=(k == 0), stop=(skip_bias and k == kd - 1))
                if not skip_bias:
                    nc.tensor.matmul(
                        ps[:], lhsT=bqkv_bf[0:1, 0:dh],
                        rhs=ones_row[0:1, 0:ch], start=False, stop=True)
                nc.vector.tensor_copy(qT_bf[:, h * ch:(h + 1) * ch], ps[:])

            # stage gathered kv into SBUF (cat layout only).
            # cat slot s<8 holds chunk s (gather0 slot s); slot s>=8 holds
            # chunk 23-s (gather1 slot s-8, natural order).
            kT_cat = gath.tile([128, nchunk, ch], BF16, tag="kT_cat")
            v_cat = gath.tile([128, nchunk, sub, dh], BF16, tag="v_cat")
            # half 0's v goes on the scalar queue (idle during AG0; the
            # diag exps are ordered ahead of it below) so both half-0
            # staging DMAs run in parallel and the static phase starts
            # earlier.
            d2h0 = None
            for hh in range(2):
                cc, kv_g = cc_insts[hh]
                src = kv_g.ap().rearrange("(r t p) c -> t p r c", t=2, p=128)
                d1 = nc.sync.dma_start(
                    kT_cat[:, cores * hh:cores * (hh + 1), :], src[0])
                v_eng = nc.scalar if hh == 0 else nc.sync
                d2 = v_eng.dma_start(
                    v_cat[:, cores * hh:cores * (hh + 1), :, :],
                    src[1].rearrange("p r (t d) -> p r t d", t=sub))
                if hh == 0:
                    d2h0 = d2
                add_dep_helper(d1.ins, cc.ins, sync=True,
                               reason="gather staging waits on collective")
                add_dep_helper(d2.ins, cc.ins, sync=True,
                               reason="gather staging waits on collective")

            # ---------------- attention (software-pipelined) ----------
            # Item k's score matmuls + exps are emitted before item k-1's
            # PV/rowsum matmuls, so the tensor queue never head-of-line
            # blocks on an exp in flight: while the scalar engine computes
            # exp(k), the tensor engine runs the next score matmuls.
            c_reg = nc.vector.partition_id()

            O1 = o1_ps.tile([128, ch], F32, tag="O1")
            rs1 = rs1_ps.tile([1, ch], F32, tag="rs1")

            n_static = cores
            ngroups = sub // gs

            class Item:
                pass

            def emit_st_exp(itm):
                """Score matmuls + exps (+ causal mask) for one item."""
                itm.ex = []
                itm.first_mm = None
                for gi in range(ngroups):
                    g = gi * gs
                    stp = st_ps.tile([128, gs * ch], F32, tag="St")
                    for ui in range(gs):
                        mm = nc.tensor.matmul(
                            stp[:, ui * ch:(ui + 1) * ch],
                            lhsT=itm.k_fn(g + ui), rhs=itm.q_ap,
                            start=True, stop=True)
                        itm.first_mm = itm.first_mm or mm
                    ex = exps.tile([128, gs * ch], BF16, tag="ex")
                    e_i = nc.scalar.activation(ex[:], stp[:], AF.Exp,
                                               scale=scale)
                    if itm.act_after is not None:
                        add_dep_helper(e_i.ins, itm.act_after, sync=False,
                                       reason="exp after bounce dma")
                    if itm.mask:
                        nc.vector.tensor_mul(
                            ex[:], ex[:],
                            masks[:, g * ch:(g + gs) * ch])
                    itm.last_exp = e_i
                    itm.ex.append(ex)

            def emit_pv_rs(itm):
                """Weighted-value + rowsum matmuls (and SBUF accumulation)."""
                o_start, o_stop = itm.startstop
                if itm.o_ps is None:
                    itm.o_ps = misc_ps.tile([128, ch], F32, tag="mps")
                    itm.rs_ps = misc_ps.tile([1, ch], F32, tag="mps")
                for gi in range(ngroups):
                    g = gi * gs
                    ex = itm.ex[gi]
                    for ui in range(gs):
                        u = g + ui
                        nc.tensor.matmul(
                            itm.o_ps[:],
                            lhsT=itm.v_fn(u),
                            rhs=ex[:, ui * ch:(ui + 1) * ch],
                            start=(o_start and u == 0),
                            stop=(o_stop and u == sub - 1))
                    for ui in range(gs):
                        u = g + ui
                        itm.last_mm = nc.tensor.matmul(
                            itm.rs_ps[0:1, :],
                            lhsT=ones_col[:, 0:1],
                            rhs=ex[:, ui * ch:(ui + 1) * ch],
                            start=(o_start and u == 0),
                            stop=(o_stop and u == sub - 1))
                if itm.acc_sl is not None:
                    sl = itm.acc_sl
                    nc.vector.tensor_add(
                        O_acc[:, sl], O_acc[:, sl], itm.o_ps[:])
                    itm.last_add = nc.vector.tensor_add(
                        rs_acc[0:1, sl], rs_acc[0:1, sl], itm.rs_ps[0:1, :])

            def mk(k_fn, v_fn, q_ap, o_ps, rs_ps, startstop, acc_sl,
                   mask=False, act_after=None, pre=None):
                itm = Item()
                itm.k_fn, itm.v_fn, itm.q_ap = k_fn, v_fn, q_ap
                itm.o_ps, itm.rs_ps = o_ps, rs_ps
                itm.startstop, itm.acc_sl = startstop, acc_sl
                itm.mask, itm.act_after, itm.pre = mask, act_after, pre
                return itm

            items = []
            for h in range(2):
                items.append(mk(
                    lambda u, h=h: kT_loc[h][:, 128 * u:128 * (u + 1)],
                    lambda u, h=h: v_loc[h][:, u, :],
                    qT_bf[:, h * ch:(h + 1) * ch],
                    None, None, (True, True),
                    slice(h * ch, (h + 1) * ch),
                    mask=True,
                    act_after=last_bounce.ins if h == 0 else None))
            for t in range(n_static):
                items.append(mk(
                    lambda u, b=t: kT_cat[:, b, 128 * u:128 * (u + 1)],
                    lambda u, b=t: v_cat[:, b, u, :],
                    qT_bf[:, ch:2 * ch],
                    O1, rs1, (t == 0, t == n_static - 1), None))
            dyn_items = []
            for tq in range(cores - 1):
                itm = mk(None, None, None, None, None, (True, True), None)
                itm.tq = tq
                dyn_items.append(itm)
                items.append(itm)

            def emit_dyn_pre(itm):
                tq = itm.tq
                isl = nc.snap((22 - tq - c_reg) >> 4,
                              donate=True, min_val=0, max_val=1)
                blk = nc.snap(tq + c_reg - (cores - 1)
                              + isl * (nchunk - 1 - c_reg),
                              donate=True, min_val=0, max_val=nchunk - 1)
                g_reg = nc.snap(blk >> 3, donate=True, min_val=0, max_val=1)
                slot = nc.snap(blk + g_reg * ((3 * cores - 1) - 2 * blk),
                               donate=True, min_val=0, max_val=nchunk - 1)
                qst = dynp.tile([128, ch], BF16, tag="qst")
                nc.vector.tensor_copy(qst[:], qT_bf[:, ds(isl * ch, ch)])
                kst = dynp.tile([128, 1, ch], BF16, tag="kst")
                nc.vector.tensor_copy(kst[:], kT_cat[:, ds(slot, 1), :])
                vst = dynp.tile([128, 1, sub, dh], BF16, tag="vst")
                nc.vector.tensor_copy(vst[:], v_cat[:, ds(slot, 1), :, :])
                itm.k_fn = lambda u: kst[:, 0, 128 * u:128 * (u + 1)]
                itm.v_fn = lambda u: vst[:, 0, u, :]
                itm.q_ap = qst[:]
                itm.acc_sl = ds(isl * ch, ch)

            # pipelined emission: st/exp of item k, then pv/rs of item k-1
            prev = None
            for k, itm in enumerate(items):
                if hasattr(itm, "tq"):
                    emit_dyn_pre(itm)
                emit_st_exp(itm)
                if prev is not None:
                    emit_pv_rs(prev)
                prev = itm
            emit_pv_rs(prev)

            # arrival gates (see kernel3 notes): marker nops pinned after the
            # diagonal work; static/dynamic reads ordered behind them.
            mark_t = nc.tensor.nop(nofuse=True, hint="kv_arrival_t")
            mark_v = nc.vector.nop(nofuse=True, hint="kv_arrival_v")
            add_dep_helper(mark_t.ins, items[1].last_mm.ins, sync=False,
                           reason="marker after diag matmuls")
            add_dep_helper(mark_v.ins, items[1].last_add.ins, sync=False,
                           reason="marker after diag adds")
            for t in range(n_static):
                add_dep_helper(items[2 + t].first_mm.ins, mark_t.ins,
                               sync=False, reason="static after gather0")
            add_dep_helper(d2h0.ins, items[1].last_exp.ins, sync=False,
                           reason="diag exps ahead of the blocking staging")

            # ---------------- epilogue ----------------
            for h in range(2):
                Ot = epip.tile([128, ch], BF16, tag="Ot")
                rs_row = epip.tile([1, ch], F32, tag="rs_row")
                if h == 1:
                    nc.vector.tensor_add(Ot[:], O_acc[:, ch:2 * ch], O1[:])
                    nc.vector.tensor_add(rs_row[:], rs_acc[0:1, ch:2 * ch],
                                         rs1[0:1, :])
                else:
                    nc.vector.tensor_copy(Ot[:], O_acc[:, 0:ch])
                    nc.vector.tensor_copy(rs_row[:], rs_acc[0:1, 0:ch])
                rs_bf = epip.tile([1, ch], BF16, tag="rs_bf")
                if not skip_bias:
                    nc.vector.tensor_copy(rs_bf[:], rs_row[:])
                for tt in range(it):
                    rsT = misc_ps.tile([128, 1], F32, tag="mps")
                    nc.tensor.matmul(
                        rsT[:],
                        lhsT=rs_row[0:1, 128 * tt:128 * (tt + 1)],
                        rhs=one_f[0:1, 0:1], start=True, stop=True)
                    rec = epip.tile([128, 1], F32, tag="rec")
                    nc.vector.reciprocal(rec[:], rsT[:])
                    osb = outpp.tile([128, d_out], F32, tag="osb")
                    for m in range(mh):
                        ops = misc_ps.tile([128, m_t], F32, tag="mps")
                        nc.tensor.matmul(
                            ops[:],
                            lhsT=Ot[:, 128 * tt:128 * (tt + 1)],
                            rhs=wout_bf[:, m * m_t:(m + 1) * m_t],
                            start=True, stop=skip_bias)
                        if not skip_bias:
                            nc.tensor.matmul(
                                ops[:],
                                lhsT=rs_bf[0:1, 128 * tt:128 * (tt + 1)],
                                rhs=bout_bf[0:1, m * m_t:(m + 1) * m_t],
                                start=False, stop=True)
                        if (tt * mh + m) % 2 == 0:
                            nc.scalar.activation(
                                osb[:, m * m_t:(m + 1) * m_t], ops[:],
                                AF.Identity, scale=rec[:, 0:1])
                        else:
                            nc.vector.tensor_scalar_mul(
                                osb[:, m * m_t:(m + 1) * m_t], ops[:],
                                rec[:, 0:1])
                    dma_eng = nc.sync if tt % 2 == 0 else nc.scalar
                    dma_eng.dma_start(
                        out[h * ch + 128 * tt: h * ch + 128 * (tt + 1), :],
                        osb[:])

    nc.compile()
    return nc


# ---------------- host side ----------------

_CACHED = {}


def _get_program(key, **kw):
    if key not in _CACHED:
        _CACHED[key] = build_program(**kw)
    return _CACHED[key]


def shard_inputs(x, w_qkv, b_qkv, w_out, b_out, cores=8):
    import ml_dtypes
    n = x.shape[0]
    nchunk = 2 * cores
    ch = n // nchunk
    sub = ch // 128
    ii = np.arange(ch)[None, :]
    jj = np.arange(128)[:, None]
    tri = np.concatenate(
        [(ii >= 128 * u + jj) for u in range(sub)],
        axis=1).astype(ml_dtypes.bfloat16)
    wq = np.ascontiguousarray(w_qkv).astype(ml_dtypes.bfloat16)
    wo = np.ascontiguousarray(w_out).astype(ml_dtypes.bfloat16)
    bq = np.ascontiguousarray(b_qkv).reshape(1, -1).astype(np.float32)
    bo = np.ascontiguousarray(b_out).reshape(1, -1).astype(np.float32)
    in_maps = []
    for c in range(cores):
        xs = np.concatenate(
            [x[ch * c: ch * (c + 1)],
             x[ch * (nchunk - 1 - c): ch * (nchunk - c)]], axis=0)
        in_maps.append({
            "xT": np.ascontiguousarray(xs.T).astype(ml_dtypes.bfloat16),
            "w_qkv": wq, "b_qkv": bq, "w_out": wo, "b_out": bo, "tri": tri,
        })
    return in_maps


def unshard_output(results, n, d_out, cores=8):
    nchunk = 2 * cores
    ch = n // nchunk
    out = np.empty((n, d_out), dtype=np.float32)
    for c in range(cores):
        o = results[c]["out"]
        out[ch * c: ch * (c + 1)] = o[:ch]
        out[ch * (nchunk - 1 - c): ch * (nchunk - c)] = o[ch:]
    return out


def kernel(x, w_qkv, b_qkv, w_out, b_out):
    from concourse.bass_utils import run_bass_kernel_spmd

    x = np.asarray(x)
    w_qkv = np.asarray(w_qkv)
    b_qkv = np.asarray(b_qkv)
    w_out = np.asarray(w_out)
    b_out = np.asarray(b_out)
    cores = 8
    n, d_in = x.shape
    d_out = w_out.shape[1]
    dh = w_out.shape[0]
    skip_bias = not (np.any(b_qkv) or np.any(b_out))
    nc = _get_program(
        (cores, n, d_in, d_out, dh, skip_bias),
        cores=cores, n=n, d_in=d_in, d_out=d_out, dh=dh,
        skip_bias=skip_bias)
    in_maps = shard_inputs(x, w_qkv, b_qkv, w_out, b_out, cores)
    res = run_bass_kernel_spmd(nc, in_maps, core_ids=list(range(cores)))
    return unshard_output(res.results, n, d_out, cores)
